# revision 2
# baseline (speedup 1.0000x reference)
"""MultiHeadDistanceLayer Trainium2 kernel, v4.

Sharding: batch b -> core b (8 cores, data parallel, no collectives).
Per core: fp8 DoubleRow projections (K=256 as 128x2 pairs, origin tile),
fp16 band matmuls (2-stacked 64-row windows, PE quadrant tiles), DVE
G-mult, ACT exp, ow32 reduce matmuls packing 4 (head, col-quarter)
results per PSUM bank, per-pair staging + output DMA.

Problem: B=8, F=256, L=2048, H=8, D=32.
  x = inputs^T [B, L, F]; q = x@Wq + bq; k = x@Wk + bk  (per-head D=32)
  att = (q.k / sqrt(D)) * prior(m - l);  prior = Gaussian(mean, std)
  p = softmax_m(att);  out[b, l, h] = sum_m p[l, m] * (m - l)

Same banded-softmax algebra as v1 (only a +-16 halo matters; out-of-band
exp(0)=1 handled by host-side constants).  Differences vs v1:

  * fp8e4m3 data path: x and the (16x prescaled) weights ship as fp8; the
    projections run as DoubleRow matmuls (K=256 as 128x2 pairs, 0.5
    cyc/row); band matmuls use DoubleRow with stride-0 pair broadcast
    (computes 2*k^T q at 0.5 cyc/row); the extra 2x and the 16x16 weight
    prescale are folded into the G table.
  * reduce matmuls use ow32 [128, 32] (4 useful cols) and tile_position
    column packing: 4 (head, col-quarter) results per PSUM bank ->
    one [128, 512] staging copy per HEAD PAIR instead of per head.
  * output: single staging tile, 2 output DMAs of the useful rows.
  * projection PSUM->SBUF copies write fp8 q/k directly (bias folded in),
    split across DVE/ACT to balance with the G-mult (DVE-only, PSUM) and
    exp (ACT-only) band passes.

"""

import numpy as np
import ml_dtypes

import concourse.bass as bass
import concourse.mybir as mybir
import concourse.tile as tile
from concourse import bacc
from concourse.bass_utils import run_bass_kernel_spmd

F32 = mybir.dt.float32
F16 = mybir.dt.float16
F8 = mybir.dt.float8e4
AF = mybir.ActivationFunctionType
ALU = mybir.AluOpType
DR = mybir.MatmulPerfMode.DoubleRow

B, F, L, H, D = 8, 256, 2048, 8, 32
HD = H * D
INV_SQRT_2PI = 1.0 / np.sqrt(2.0 * 3.1415926)

WW = 16          # halo; band half-width needed is ~13
GROUP = 32       # l-columns per band matmul
WIN = 64         # window rows per stacked group
NB = L // 64     # 32 blocks per head
MC = 2           # m-chunks (128 q/k channels each)
NQ = 4           # x quarters
PN = L // NQ     # 512
WSC = 16.0       # weight prescale (fp8 dynamic range)
# fp8 matmuls fail at non-origin tile_position on this stack (ISA check for
# DoubleRow, runtime error without) -> band matmuls run in fp16
SSC = WSC * WSC  # band psum = SSC * s

# packed setup layout (fp32 cols): w8-m0 | ow32 | bqr | bkr | GT | w8-m1
# (m0 weights + small tables ship in the first DMA so projections and
# copies unblock early; m1 weights trail in a second DMA)
C_W0 = 0
C_OW = C_W0 + 128
C_BQ = C_OW + 32
C_BK = C_BQ + MC
C_GT = C_BK + MC
C_W1 = C_GT + GROUP
S_TOT = C_W1 + 128

# knobs
NWARM = 8
# which proj copy units go on DVE (True) vs ACT: indexed (qk, m, jq);
# qk=0 is q, qk=1 is k.  Default: q on DVE, k on ACT, jq-granular.
PROJ_COPY_DVE = {(0, m, j): m == 0 for m in range(MC) for j in range(NQ)}
PROJ_COPY_DVE.update({(1, m, j): False for m in range(MC) for j in range(NQ)})
# exp granularity per pair: True = one [128,2048] exp, False = per head
EXP_PAIR = {0: False, 1: False, 2: False, 3: False}
# staging copies per pair: True -> DVE
STAGE_DVE = {0: True, 1: False, 2: True, 3: False}


def build_nc(stages="full"):
    nc = bacc.Bacc("TRN2", target_bir_lowering=False, debug=False)

    x_d = nc.dram_tensor("x", [F, L], F8, kind="ExternalInput")
    s_d = nc.dram_tensor("setup", [128, S_TOT], F32, kind="ExternalInput")
    zn_d = nc.dram_tensor("zn", [2, 4, H, 512], F32, kind="ExternalOutput")

    with tile.TileContext(nc) as tc:
        with (
            tc.tile_pool(name="const", bufs=1) as constp,
            tc.tile_pool(name="xin", bufs=1) as xinp,
            tc.tile_pool(name="qk", bufs=1) as qkp,
            tc.tile_pool(name="pwarm", bufs=1, space="PSUM") as pwarmp,
        ):
            # ---- x quarter DMAs issued first: j0 on the ACT HWDGE ring,
            # j2 on SP behind the m0 setup, j1/j3 on the Pool SWDGE ring ----
            x_q = []
            for j in range(NQ):
                x_q.append(xinp.tile([128, 2, PN], F8, tag=f"x{j}",
                                     name=f"x{j}"))

            def x_dma(j, eng):
                eng.dma_start(
                    x_q[j][:],
                    x_d.ap()[:, j * PN:(j + 1) * PN].rearrange(
                        "(i p) l -> p i l", p=128),
                )

            # wz memset first on DVE so the PE warmups start immediately
            wz = constp.tile([128, 384], F16, tag="wz")
            nc.vector.memset(wz[:], 0.0)
            cst = constp.tile([128, S_TOT], F32, tag="cst")
            x_dma(0, nc.sync)
            nc.gpsimd.dma_start(cst[:], s_d.ap()[:])
            x_dma(2, nc.scalar)
            x_dma(1, nc.gpsimd)
            x_dma(3, nc.sync)
            # ---- PE warmup (never-closed pool: no bank anti-deps) + Exp
            # table preload, all while the inputs DMA ----
            wps = pwarmp.tile([128, 512], F32, tag="wps", name="wps")
            for i in range(NWARM):
                nc.tensor.matmul(
                    wps[:, 0:256], wz[:, 0:128], wz[:, 0:256], start=True,
                    stop=True, skip_group_check=True,
                )
            pre = constp.tile([128, 1], F16, tag="pre")
            nc.scalar.activation(pre[:], wz[:, 0:1], AF.Exp)

            w8m = [
                cst[:, C_W0:C_W0 + 128].bitcast(F8).rearrange(
                    "p (qk i m) -> p qk i m", qk=2, i=2),
                cst[:, C_W1:C_W1 + 128].bitcast(F8).rearrange(
                    "p (qk i m) -> p qk i m", qk=2, i=2),
            ]
            ow64 = cst[:, C_OW:C_OW + 32].bitcast(F16)      # [128, 64]
            bqr = cst[:, C_BQ:C_BQ + MC]
            bkr = cst[:, C_BK:C_BK + MC]
            g32 = cst[:, C_GT:C_GT + GROUP]
            gT = g32[:, None, :].broadcast_to((128, NB, GROUP))

            qT = [qkp.tile([128, L], F16, tag=f"qT{m}", name=f"qT{m}")
                  for m in range(MC)]
            kT = [qkp.tile([128, L + 2 * WW], F16, tag=f"kT{m}",
                           name=f"kT{m}") for m in range(MC)]
            for m in range(MC):
                nc.gpsimd.memset(kT[m][:, 0:WW], 0.0)
                nc.gpsimd.memset(kT[m][:, L + WW:L + 2 * WW], 0.0)

            znall = qkp.tile([128, 4096], F32, tag="znall")

            if stages == "loads":
                dummy = qkp.tile([128, 512], F32, tag="dummy")
                nc.vector.memset(dummy[:], 0.0)
                for a in range(2):
                    nc.sync.dma_start(zn_d.ap()[a], dummy[0:4, 0:512]
                                      [:, None, :].broadcast_to((4, H, 512)))
                nc.compile()
                return nc

            # ---- all PSUM pools open together: disjoint bank regions so
            # proj copies, band matmuls and reduces pipeline freely.
            # proj+znred share one [128,512]x3 pool (3 banks), sT 2x2 banks.
            with (
                tc.tile_pool(name="pproj", bufs=3, space="PSUM") as pprojp,
                tc.tile_pool(name="pband", bufs=2, space="PSUM") as pbandp,
                tc.tile_pool(name="att", bufs=2) as attp,
                tc.tile_pool(name="pexp", bufs=2) as pexpp,
            ):
                units = [(qk, m, j) for m in range(MC)
                         for j in (0, 2, 1, 3) for qk in (1, 0)]
                for u, (qk, m, j) in enumerate(units):
                    ps = pprojp.tile([128, PN], F32, tag="pp", name=f"pp{u}")
                    lhsT = w8m[m][:, qk, :, :]              # [128, 2, 128]
                    nc.tensor.matmul(ps[:], lhsT, x_q[j][:], start=True,
                                     stop=True, perf_mode=DR)
                    if stages == "projmm":
                        continue
                    if qk == 1:
                        dest = kT[m][:, WW + j * PN: WW + (j + 1) * PN]
                        bias = bkr
                    else:
                        dest = qT[m][:, j * PN:(j + 1) * PN]
                        bias = bqr
                    if PROJ_COPY_DVE[(qk, m, j)]:
                        nc.vector.tensor_scalar(
                            dest, ps[:], bias[:, m:m + 1], None, op0=ALU.add)
                    else:
                        nc.scalar.activation(
                            dest, ps[:], AF.Identity, bias=bias[:, m:m + 1])

                if stages in ("proj", "projmm"):
                    dummy = qkp.tile([128, 512], F32, tag="dummy")
                    nc.vector.memset(dummy[:], 0.0)
                    for a in range(2):
                        nc.sync.dma_start(
                            zn_d.ap()[a], dummy[0:4, 0:512]
                            [:, None, :].broadcast_to((4, H, 512)))
                    nc.compile()
                    return nc

                znred = {}
                pt_of = {}

                def emit_reduce_head(h):
                    znb = pwarmp.tile([128, 512], F32, tag="wps",
                                      name=f"znb{h}")
                    znred[h] = znb
                    ptp = pt_of[h // 2]
                    hl = h % 2
                    for qq in range(2):
                        nc.tensor.matmul(
                            znb[64 * qq:64 * qq + 64, :],
                            ow64,
                            ptp[:, hl * 1024 + qq * 512:
                                hl * 1024 + (qq + 1) * 512],
                            start=True, stop=True,
                            tile_position=(0, 64 * qq),
                        )

                def emit_reduce(pr):
                    znb = pwarmp.tile([128, 512], F32, tag="wps",
                                      name=f"znb{pr}")
                    znred[pr] = znb
                    ptp = pt_of[pr]
                    for hl in range(2):
                        for qq in range(2):
                            s = 2 * hl + qq
                            nc.tensor.matmul(
                                znb[32 * s:32 * s + 32, :],
                                ow32,
                                ptp[:, hl * 1024 + qq * 512:
                                    hl * 1024 + (qq + 1) * 512],
                                start=True, stop=True,
                                tile_position=(0, 32 * s),
                            )

                def emit_stage(h):
                    dst = znall[:, h * 512:(h + 1) * 512]
                    if STAGE_DVE[h % 4]:
                        nc.vector.tensor_copy(dst, znred[h][:])
                    else:
                        nc.scalar.copy(dst, znred[h][:])

                att = pt = None
                for h in range(H):
                    m = h // 4
                    hp = 32 * (h % 4)
                    sT = pbandp.tile([128, 1024], F32, tag="sT", name=f"sT{h}")
                    for c2 in range(NB):
                        for g in range(2):
                            base = 64 * c2 + 32 * g
                            lhsT = kT[m][hp:hp + 32, base:base + WIN]
                            rhs = qT[m][hp:hp + 32, base:base + GROUP]
                            nc.tensor.matmul(
                                sT[64 * g:64 * g + WIN,
                                   GROUP * c2:GROUP * (c2 + 1)],
                                lhsT, rhs, start=True, stop=True,
                                tile_position=(hp, 64 * g),
                            )
                    if stages == "band":
                        continue
                    if h % 2 == 0:
                        att = attp.tile([128, 2048], F16, tag="att",
                                        name=f"att{h // 2}")
                    nc.vector.tensor_tensor(
                        att[:, (h % 2) * 1024:(h % 2 + 1) * 1024].rearrange(
                            "p (b i) -> p b i", b=NB),
                        sT[:].rearrange("p (b i) -> p b i", b=NB),
                        gT, op=ALU.mult)
                    if stages == "noexp":
                        continue
                    if h % 2 == 0:
                        pt = pexpp.tile([128, 2048], F16, tag="pt",
                                        name=f"pt{h // 2}")
                        pt_of[h // 2] = pt
                    if EXP_PAIR[h // 2]:
                        if h % 2 == 1:
                            nc.scalar.activation(pt[:], att[:], AF.Exp)
                    else:
                        half = slice((h % 2) * 1024, (h % 2 + 1) * 1024)
                        nc.scalar.activation(pt[:, half], att[:, half], AF.Exp)
                    # per-head reduce + staging right after this head's exp
                    emit_reduce_head(h)
                    emit_stage(h)
                if stages in ("band", "noexp"):
                    dummy = qkp.tile([128, 512], F32, tag="dummy")
                    nc.vector.memset(dummy[:], 0.0)
                    for a in range(2):
                        nc.sync.dma_start(
                            zn_d.ap()[a], dummy[0:4, 0:512]
                            [:, None, :].broadcast_to((4, H, 512)))
                else:
                    nc.sync.dma_start(
                        zn_d.ap()[0],
                        znall[0:4, :].rearrange("p (hh c) -> p hh c", hh=H))
                    nc.sync.dma_start(
                        zn_d.ap()[1],
                        znall[64:68, :].rearrange("p (hh c) -> p hh c", hh=H))
    nc.compile()
    return nc


_NC_CACHE = {}


def _get_nc():
    if "nc" not in _NC_CACHE:
        _NC_CACHE["nc"] = build_nc()
    return _NC_CACHE["nc"]


def _host_consts(prior_mean, prior_std):
    mu = float(np.asarray(prior_mean).reshape(-1)[0])
    sd = float(np.asarray(prior_std).reshape(-1)[0])
    # g32 block [128, 32]: rows j in [0,64) x cols i: d = (j - WW) - i;
    # rows 64..128 repeat.  Scaled by rsqrt(D)/SSC for the fp8 data path.
    j = np.arange(WIN)
    i = np.arange(GROUP)
    d = j[:, None] - WW - i[None, :]
    prior = (INV_SQRT_2PI / sd) * np.exp(
        -0.5 * (d.astype(np.float64) - mu) ** 2 / sd ** 2
    )
    gA = (prior * (float(D) ** -0.5) / SSC).astype(np.float32)
    g32 = np.concatenate([gA, gA], axis=0)
    # ow64 [128, 64] fp16: col0 = 1(p<64); col1 = (p-16)(p<64);
    # col2 = 1(p>=64); col3 = (p-80)(p>=64); cols 4:64 zero
    p = np.arange(128)
    ow = np.zeros((128, 64), np.float16)
    ow[:, 0] = (p < 64).astype(np.float16)
    ow[:, 1] = np.where(p < 64, p - WW, 0).astype(np.float16)
    ow[:, 2] = (p >= 64).astype(np.float16)
    ow[:, 3] = np.where(p >= 64, p - 64 - WW, 0).astype(np.float16)
    return g32, ow


def _pack_setup(Wq, Wk, bq, bk, prior_mean, prior_std):
    g32, ow = _host_consts(prior_mean, prior_std)

    def pack_bytes(b):  # [128, 4n] fp8 bytes -> [128, n] fp32 bit-pack
        wb = b.view(np.uint8).astype(np.uint32)
        return (wb[:, 0::4] | (wb[:, 1::4] << 8) | (wb[:, 2::4] << 16)
                | (wb[:, 3::4] << 24)).view(np.float32)

    cst = np.zeros((128, S_TOT), np.float32)
    for mc, col in ((0, C_W0), (1, C_W1)):
        w8 = np.zeros((128, 2, 2, 128), ml_dtypes.float8_e4m3)
        for qk, W in enumerate((Wq, Wk)):
            Ws = (np.asarray(W, np.float32) * WSC).astype(
                ml_dtypes.float8_e4m3)
            for i in range(2):
                w8[:, qk, i, :] = Ws[i * 128:(i + 1) * 128,
                                     mc * 128:(mc + 1) * 128]
        cst[:, col:col + 128] = pack_bytes(w8.reshape(128, 512))
    pairs = ow.view(np.uint16).reshape(128, 32, 2)
    cst[:, C_OW:C_OW + 32] = (
        pairs[:, :, 0].astype(np.uint32)
        | (pairs[:, :, 1].astype(np.uint32) << 16)
    ).view(np.float32)
    cst[:, C_BQ:C_BQ + MC] = np.asarray(bq, np.float32).reshape(MC, 128).T * WSC
    cst[:, C_BK:C_BK + MC] = np.asarray(bk, np.float32).reshape(MC, 128).T * WSC
    cst[:, C_GT:C_GT + GROUP] = g32
    return np.ascontiguousarray(cst)


def _make_in_maps(inputs, Wq, bq, Wk, bk, prior_mean, prior_std):
    x8 = np.ascontiguousarray(
        np.asarray(inputs, dtype=np.float32).astype(ml_dtypes.float8_e4m3))
    cst = _pack_setup(Wq, Wk, bq, bk, prior_mean, prior_std)
    return [{"x": x8[b], "setup": cst} for b in range(B)]


def _assemble(zn):
    """zn: [2, 4, H, 512] per core -> out [L, H] fp32.

    zn[qq, r, h, col]: r = 0:SPA 1:SWA 2:SPB 3:SWB for column-quarter qq.
    x = 512*qq + col; c2 = x//32, i = x%32; A: l = 64*c2+i; B: +32.
    """
    x = np.arange(L // 2)
    qq = x // 512
    col = x % 512
    sp = np.empty((H, L), np.float64)
    sw = np.empty((H, L), np.float64)
    c2 = x // GROUP
    i = x % GROUP
    lA = 64 * c2 + i
    lB = lA + 32
    for h in range(H):
        spa = zn[qq, 0, h, col]
        swa = zn[qq, 1, h, col]
        spb = zn[qq, 2, h, col]
        swb = zn[qq, 3, h, col]
        sp[h, lA] = spa
        sp[h, lB] = spb
        sw[h, lA] = swa
        sw[h, lB] = swb
    lidx = np.arange(L, dtype=np.float64)
    i_of_l = lidx % 64 % 32
    csum = float(WIN * (WIN - 1) / 2 - WW * WIN)           # 992
    zc = sp - WIN
    ncv = sw - csum - i_of_l[None, :] * zc
    tl = L * (L - 1) / 2.0 - lidx * float(L)
    out = (tl[None, :] + ncv) / (float(L) + zc)
    return np.ascontiguousarray(out.T.astype(np.float32))  # [L, H]


def run(in_maps, **kw):
    return run_bass_kernel_spmd(_get_nc(), in_maps, core_ids=list(range(8)),
                                **kw)


def kernel(inputs, Wq, bq, Wk, bk, prior_mean, prior_std):
    in_maps = _make_in_maps(inputs, Wq, bq, Wk, bk, prior_mean, prior_std)
    res = run(in_maps)
    return np.stack([_assemble(res.results[b]["zn"]) for b in range(B)],
                    axis=0)
